# revision 1
# baseline (speedup 1.0000x reference)
"""LocalAutoCorr2D Trainium2 kernel.

out[b,c,i,j,dy,dx] = sum_{y,x valid} x[b,c,4i+y,4j+x] * x[b,c,4i+y+sy,4j+x+sx]
with (sy,sx) = (dy-4, dx-4), windows 8x8 at stride 4 on a 96x96 image,
zero-padded at window boundaries.

Strategy (per core, batch-sharded over 8 cores):
  - out[s] == out[-s] (autocorr symmetry) -> only 40 canonical shift classes.
  - For each canonical shift (sy>=0, sx): product Q = x .* shift(x) on the
    Vector engine (fp16, 2x mode), with h on partitions so the vertical
    box-sum can run on the Tensor engine as a 0/1-weight matmul; the
    horizontal box-sum is folded into PSUM accumulation across <=8 matmuls
    whose rhs APs are column-shifted strided views of Q.
  - Vertical shifts are pre-materialized as partition-shifted SBUF copies
    (DMA); odd horizontal shifts get +1-column-shifted copies so every
    product op keeps 4B alignment for the DVE 2x perf mode.
"""

import functools
import os
import sys

import numpy as np

sys.path.insert(0, "/opt/trn_rl_repo")

import concourse.bass as bass  # noqa: E402
import concourse.bacc as bacc  # noqa: E402
import concourse.mybir as mybir  # noqa: E402
from concourse import bass_utils  # noqa: E402
from concourse.tile import TileContext  # noqa: E402

B, C, H, W = 8, 64, 96, 96
KH = KW = 8
SH = SW = 4
NH = NW = 23
NCORES = 8
CW = C * W  # 6144 flat (c,w) columns
PAD = 4  # column padding so sx in [-4,4] offsets stay in-tile

fp32 = mybir.dt.float32
fp16 = mybir.dt.float16


def _canonical_cells():
    """Map canonical shift (sy>=0, sx) -> list of output cells (dy,dx)."""
    cells = {}
    for dy in range(8):
        for dx in range(8):
            sy, sx = dy - 4, dx - 4
            key = (sy, sx) if (sy > 0 or (sy == 0 and sx >= 0)) else (-sy, -sx)
            cells.setdefault(key, []).append((dy, dx))
    assert len(cells) == 40
    return cells


def _amat_np():
    """Vertical box-sum matrices, stacked: A[h, sy*23+i] = 1 if 0<=h-4i<8-sy."""
    a = np.zeros((H, 5 * NH), np.float16)
    for sy in range(5):
        for i in range(NH):
            a[4 * i : 4 * i + 8 - sy, sy * NH + i] = 1.0
    return a


C_CHUNKS = [(0, 22), (22, 43), (43, 64)]  # N = 506/483/483 <= 512 per matmul


def build_nc():
    nc = bacc.Bacc()
    x_dram = nc.dram_tensor("x", [C, H, W], fp32, kind="ExternalInput")
    amat_dram = nc.dram_tensor("amat", [H, 5 * NH], fp16, kind="ExternalInput")
    out_dram = nc.dram_tensor("out", [8, 8, NH, CW_OUT := C * NW], fp32,
                              kind="ExternalOutput")

    cells = _canonical_cells()
    # order: by sy so early shifts only need T00, copies land meanwhile
    order = sorted(cells.keys(), key=lambda s: (s[0], abs(s[1])))

    with TileContext(nc) as tc:
        with (
            tc.tile_pool(name="const", bufs=1) as cpool,
            tc.tile_pool(name="xstage", bufs=1) as xpool,
            tc.tile_pool(name="tcop", bufs=1) as tpool,
            tc.tile_pool(name="q", bufs=2) as qpool,
            tc.tile_pool(name="o", bufs=2) as opool,
            tc.tile_pool(name="ps", bufs=2, space="PSUM") as ppool,
        ):
            amat_t = cpool.tile([H, 5 * NH], fp16)
            nc.gpsimd.dma_start(amat_t, amat_dram[:, :])

            xr = x_dram[:, :, :].rearrange("c h w -> h c w")

            # T tiles: T[(sy,p)][r, PAD+k] = x16[r+sy, k+p]
            tt = {}
            for sy in range(5):
                for p in range(2):
                    if (sy, p) == (0, 0):
                        t = tpool.tile([H, PAD + CW + PAD], fp16, name="T00")
                    else:
                        t = tpool.tile([H, PAD + CW + PAD], fp16,
                                       name=f"T{sy}{p}")
                    tt[(sy, p)] = t
                    nc.vector.memset(t[:, 0:PAD], 0.0)
                    nc.vector.memset(t[:, PAD + CW : PAD + CW + PAD], 0.0)

            t00 = tt[(0, 0)]
            # load fp32 in 2 chunks, then DVE-convert to fp16 in 2 halves so
            # every downstream consumer of t00 has a single (same-engine)
            # producer -- avoids blowing the per-instruction sem-wait limit.
            x32 = xpool.tile([H, CW], fp32)
            for k in range(2):
                sl = slice(k * 3072, (k + 1) * 3072)
                nc.gpsimd.dma_start(x32[:, sl], xr[:, k * 32 : (k + 1) * 32, :])
                nc.vector.tensor_copy(
                    t00[:, PAD + k * 3072 : PAD + (k + 1) * 3072], x32[:, sl]
                )
            # shifted copies via SBUF->SBUF DMA
            for (sy, p), t in tt.items():
                if (sy, p) == (0, 0):
                    continue
                nc.gpsimd.dma_start(
                    t[0 : H - sy, PAD : PAD + CW - p],
                    t00[sy:H, PAD + p : PAD + CW],
                )

            for (sy, sx) in order:
                p = sx & 1
                hv = H - sy
                q = qpool.tile([H, CW], fp16, tag="q")
                off = PAD + sx - p
                nc.vector.tensor_mul(
                    q[0:hv, :],
                    t00[0:hv, PAD : PAD + CW],
                    tt[(sy, p)][0:hv, off : off + CW],
                )
                qv = q.rearrange("h (c w) -> h c w", c=C)
                a_k = amat_t[0:hv, sy * NH : (sy + 1) * NH]
                xlist = list(range(max(0, -sx), 8 - max(0, sx)))
                o_t = opool.tile([NH, C * NW], fp32, tag="o")
                for ci, (c0, c1) in enumerate(C_CHUNKS):
                    pt = ppool.tile([NH, (c1 - c0) * NW], fp32, tag=f"ps{ci}")
                    for xi, xx in enumerate(xlist):
                        rhs = qv[0:hv, c0:c1, xx : xx + 4 * NW - 3 : 4]
                        nc.tensor.matmul(
                            pt, a_k, rhs,
                            start=(xi == 0), stop=(xi == len(xlist) - 1),
                        )
                    nc.scalar.copy(o_t[:, c0 * NW : c1 * NW], pt)
                for (dy, dx) in cells[(sy, sx)]:
                    nc.gpsimd.dma_start(out_dram[dy, dx], o_t)

    if not nc.is_finalized():
        nc.finalize()
    return nc


@functools.lru_cache(maxsize=1)
def _get_nc():
    return build_nc()


def kernel(**inputs) -> np.ndarray:
    x = np.asarray(inputs["x"], dtype=np.float32)
    assert x.shape == (B, C, H, W)
    amat = _amat_np()
    nc = _get_nc()
    in_maps = [
        {"x": np.ascontiguousarray(x[b]), "amat": amat} for b in range(NCORES)
    ]
    res = bass_utils.run_bass_kernel_spmd(
        nc, in_maps, core_ids=list(range(NCORES)),
        trace=bool(int(os.environ.get("KERNEL_TRACE", "0"))),
    )
    outs = np.stack([r["out"] for r in res.results])  # [B, dy, dx, i, (c j)]
    outs = outs.reshape(B, 8, 8, NH, C, NW)
    # -> [B, c, i, j, dy, dx]
    full = outs.transpose(0, 4, 3, 5, 1, 2)
    return np.ascontiguousarray(full).astype(np.float32)


if __name__ == "__main__":
    rng = np.random.default_rng(0)
    x = rng.standard_normal((B, C, H, W), dtype=np.float32)
    y = kernel(x=x)
    print("out", y.shape, y.dtype, float(np.abs(y).max()))



# revision 5
# speedup vs baseline: 1.1289x; 1.1289x over previous
"""LocalAutoCorr2D Trainium2 kernel (v2: packed phase-major layout).

out[b,c,i,j,dy,dx] = sum_{y,x valid} x[b,c,4i+y,4j+x] * x[b,c,4i+y+sy,4j+x+sx]
with (sy,sx) = (dy-4, dx-4), 8x8 windows at stride 4 on a 96x96 image,
zero-padded at window boundaries.  Batch-sharded over 8 cores.

Layout: superimage rows g = 96*(c div 16) + h, g in [0,384); partition
p = g mod 128, free dim = (beta=g div 128, r=w mod 4, ap=w div 4 + 1,
c16=c mod 16) with zero pad columns at ap=0 and ap=25.  FD = 3*4*26*16
= 4992 fp16 elements per partition.

Per canonical shift (40 classes after out[s]==out[-s] symmetry):
  - product Q = X0 .* T_sy (T_sy = partition-shifted copy, sy rows up)
    as 2-3 contiguous-slice DVE/GpSimd ops: the phase-major layout turns
    the horizontal shift sx into a flat free-dim offset.
  - vertical 8-row box-sum: matmul with 0/1 A-matrices over partitions;
    horizontal box-sum folded into PSUM accumulation over xx offsets.
    Channel-group c4 = col-tile group (tile_position=(0,32*c4)) -> 4
    concurrent accumulation chains; N=384 contiguous rhs per matmul.
  - PSUM -> SBUF on Scalar (A padded to M=32 so all 128 rows are real),
    DMA out 40 canonical cells only; mirrors assembled host-side.
"""

import functools
import os
import sys

import numpy as np

sys.path.insert(0, "/opt/trn_rl_repo")

import concourse.bass as bass  # noqa: E402,F401
import concourse.bacc as bacc  # noqa: E402
import concourse.mybir as mybir  # noqa: E402
from concourse import bass_utils  # noqa: E402
from concourse.tile import TileContext  # noqa: E402

B, C, H, W = 8, 64, 96, 96
NH = NW = 23
NCORES = 8
P = 128
NB = 3            # beta blocks of 128 superrows
C4, C16 = 4, 16   # channel groups x channels-per-group
APD = 26          # padded a-dim: ap = a+1, zeros at ap=0,25
FR = APD * C16    # 416: elems per (phase r) block
FB = 4 * FR       # 1664: elems per beta block
FD = NB * FB      # 4992: total free dim

fp32 = mybir.dt.float32
fp16 = mybir.dt.float16

# (c4, beta) combos that hold rows of channel-group c4
CB_LIST = [(0, 0), (1, 0), (1, 1), (2, 1), (2, 2), (3, 2)]
# per-c4 beta list
C4_BETAS = {0: [0], 1: [0, 1], 2: [1, 2], 3: [2]}
# load pieces: (beta, p0, p1, c4, h0)
LOAD_PIECES = [
    (0, 0, 96, 0, 0), (0, 96, 128, 1, 0), (1, 0, 64, 1, 32),
    (1, 64, 128, 2, 0), (2, 0, 32, 2, 64), (2, 32, 128, 3, 0),
]

# shifts whose products run on GpSimd instead of Vector
GPSIMD_SHIFTS = frozenset({(1, 2), (2, -2), (2, 2), (3, -2), (3, 2), (4, 2)})


def _canonical_cells():
    """Map canonical shift (sy>=0, sx) -> list of output cells (dy,dx)."""
    cells = {}
    for dy in range(8):
        for dx in range(8):
            sy, sx = dy - 4, dx - 4
            key = (sy, sx) if (sy > 0 or (sy == 0 and sx >= 0)) else (-sy, -sx)
            cells.setdefault(key, []).append((dy, dx))
    assert len(cells) == 40
    return cells


CELLS = _canonical_cells()
ORDER = sorted(CELLS.keys(), key=lambda s: (s[0], abs(s[1])))


def _amat_np():
    """A[p, ((sy*6 + cbi)*32 + i)] = 1 if superrow g=128*beta+p belongs to
    channel-group c4 and its h is in vertical window i for shift sy."""
    a = np.zeros((P, 5 * len(CB_LIST) * 32), np.float16)
    for sy in range(5):
        for cbi, (c4, beta) in enumerate(CB_LIST):
            for p in range(P):
                g = 128 * beta + p
                if g // 96 != c4:
                    continue
                h = g % 96
                for i in range(NH):
                    if 0 <= h - 4 * i < 8 - sy:
                        a[p, (sy * 6 + cbi) * 32 + i] = 1.0
    return a


def build_nc():
    nc = bacc.Bacc()
    x_dram = nc.dram_tensor("x", [C, H, W], fp32, kind="ExternalInput")
    amat_dram = nc.dram_tensor("amat", [P, 5 * 6 * 32], fp16,
                               kind="ExternalInput")
    out_dram = nc.dram_tensor("out", [40, C4, NH, NH * C16], fp32,
                              kind="ExternalOutput")

    with TileContext(nc) as tc:
        with (
            tc.tile_pool(name="const", bufs=1) as cpool,
            tc.tile_pool(name="stage", bufs=1) as spool,
            tc.tile_pool(name="xt", bufs=1) as tpool,
            tc.tile_pool(name="q", bufs=4) as qpool,
            tc.tile_pool(name="o", bufs=2) as opool,
            tc.tile_pool(name="ps", bufs=3, space="PSUM") as ppool,
        ):
            amat_t = cpool.tile([P, 5 * 6 * 32], fp16)
            nc.sync.dma_start(amat_t, amat_dram[:, :])

            # ---- load x into staging, layout [p, (beta, c16, w)] fp32
            stage = spool.tile([P, NB * C16 * W], fp32)
            for beta, p0, p1, c4, h0 in LOAD_PIECES:
                src = x_dram[16 * c4:16 * (c4 + 1), h0:h0 + (p1 - p0), :]
                nc.sync.dma_start(
                    stage[p0:p1, beta * C16 * W:(beta + 1) * C16 * W],
                    src.rearrange("c h w -> h c w"),
                )

            # ---- X0: phase-major packed fp16, zero-padded
            x0 = tpool.tile([P, FD], fp16)
            nc.vector.memset(x0, 0.0)
            x0v = x0.rearrange("p (b r a c) -> p b r a c", b=NB, r=4, a=APD)
            stgv = stage.rearrange("p (b c w) -> p b w c", b=NB, c=C16)
            for beta in range(NB):
                for r in range(4):
                    nc.scalar.copy(
                        x0v[:, beta, r, 1:25, :],
                        stgv[:, beta, r::4, :],
                    )

            # ---- vertical-shift copies T_sy (superrow g -> g+sy)
            tt = {0: x0}
            for sy in range(1, 5):
                t = tpool.tile([P, FD], fp16, name=f"T{sy}")
                tt[sy] = t
                nc.gpsimd.memset(t[96:P, 2 * FB:3 * FB], 0.0)
                nc.sync.dma_start(t[0:P - sy, :], x0[sy:P, :])
                nc.sync.dma_start(t[P - sy:P, 0:2 * FB], x0[0:sy, FB:3 * FB])

            # ---- per-shift: product -> col-tiled matmuls -> copy -> DMA out
            for ks, (sy, sx) in enumerate(ORDER):
                eng = nc.gpsimd if (sy, sx) in GPSIMD_SHIFTS else nc.vector
                q = qpool.tile([P, FD], fp16, tag="q")
                q3 = q.rearrange("p (b f) -> p b f", b=NB)
                x03 = x0.rearrange("p (b f) -> p b f", b=NB)
                t3 = tt[sy].rearrange("p (b f) -> p b f", b=NB)

                def prod(o0, o1, i0):
                    eng.tensor_mul(
                        q3[:, :, o0:o1],
                        x03[:, :, o0:o1],
                        t3[:, :, i0:i0 + (o1 - o0)],
                    )

                if sx == 4:  # lambda=1 for all r; trim OOB tail (ap=25, r=3)
                    prod(0, 4 * FR - C16, C16)
                    eng.memset(q3[:, :, 4 * FR - C16:4 * FR], 0.0)
                elif sx >= 0:
                    prod(0, (4 - sx) * FR, sx * FR)
                    if sx > 0:
                        prod((4 - sx) * FR, 4 * FR, C16)
                elif sx > -4:
                    s = -sx
                    prod(0, s * FR, (4 - s) * FR - C16)
                    prod(s * FR, 4 * FR, 0)
                else:  # sx == -4: lambda=-1 for all r; trim OOB head (ap=0, r=0)
                    eng.memset(q3[:, :, 0:C16], 0.0)
                    prod(C16, FR, 0)
                    prod(FR, 4 * FR, FR - C16)

                # matmuls: 4 col-tiled chains (one per c4), interleaved
                # across col-groups so the PE quadrants run concurrently
                xlist = list(range(max(0, -sx), 8 - max(0, sx)))
                pt = ppool.tile([P, 384], fp32, tag="ps")
                for xi, xx in enumerate(xlist):
                    rx, jjx = xx & 3, xx >> 2
                    rhs_off = rx * FR + (jjx + 1) * C16
                    for cbi, (c4, bb) in enumerate(CB_LIST):
                        a_k = amat_t[:, (sy * 6 + cbi) * 32:
                                     (sy * 6 + cbi) * 32 + 32]
                        nc.tensor.matmul(
                            pt[32 * c4:32 * c4 + 32, :], a_k,
                            q3[:, bb, rhs_off:rhs_off + 384],
                            start=(xi == 0 and bb == C4_BETAS[c4][0]),
                            stop=(xi == len(xlist) - 1
                                  and bb == C4_BETAS[c4][-1]),
                            tile_position=(0, 32 * c4),
                            skip_group_check=True,
                        )

                o_t = opool.tile([P, 384], fp32, tag="o")
                nc.scalar.copy(o_t, pt)
                for c4 in range(C4):
                    nc.sync.dma_start(
                        out_dram[ks, c4],
                        o_t[32 * c4:32 * c4 + 23, 0:NH * C16],
                    )

    if not nc.is_finalized():
        nc.finalize()
    return nc


@functools.lru_cache(maxsize=1)
def _get_nc():
    return build_nc()


def _run(x, trace=False):
    amat = _amat_np()
    nc = _get_nc()
    in_maps = [
        {"x": np.ascontiguousarray(x[b]), "amat": amat} for b in range(NCORES)
    ]
    return bass_utils.run_bass_kernel_spmd(
        nc, in_maps, core_ids=list(range(NCORES)), trace=trace,
    )


def kernel(**inputs) -> np.ndarray:
    x = np.asarray(inputs["x"], dtype=np.float32)
    assert x.shape == (B, C, H, W)
    res = _run(x, trace=bool(int(os.environ.get("KERNEL_TRACE", "0"))))
    outs = np.stack([r["out"] for r in res.results])  # [B, 40, c4, i, (j c16)]
    blk = outs.reshape(B, 40, C4, NH, NH, C16)
    blk = blk.transpose(0, 1, 2, 5, 3, 4).reshape(B, 40, C, NH, NH)
    full = np.empty((B, C, NH, NH, 8, 8), np.float32)
    for ks, key in enumerate(ORDER):
        for dy, dx in CELLS[key]:
            full[:, :, :, :, dy, dx] = blk[:, ks]
    return full


if __name__ == "__main__":
    rng = np.random.default_rng(0)
    x = rng.standard_normal((B, C, H, W), dtype=np.float32)
    y = kernel(x=x)
    print("out", y.shape, y.dtype, float(np.abs(y).max()))


# revision 12
# speedup vs baseline: 1.4412x; 1.2766x over previous
"""LocalAutoCorr2D Trainium2 kernel (v2: packed phase-major layout).

out[b,c,i,j,dy,dx] = sum_{y,x valid} x[b,c,4i+y,4j+x] * x[b,c,4i+y+sy,4j+x+sx]
with (sy,sx) = (dy-4, dx-4), 8x8 windows at stride 4 on a 96x96 image,
zero-padded at window boundaries.  Batch-sharded over 8 cores.

Layout: superimage rows g = 96*(c div 16) + h, g in [0,384); partition
p = g mod 128, free dim = (beta=g div 128, r=w mod 4, ap=w div 4 + 1,
c16=c mod 16) with zero pad columns at ap=0 and ap=25.  FD = 3*4*26*16
= 4992 fp16 elements per partition.

Per canonical shift (40 classes after out[s]==out[-s] symmetry):
  - product Q = X0 .* T_sy (T_sy = partition-shifted copy, sy rows up)
    as 2-3 contiguous-slice DVE/GpSimd ops: the phase-major layout turns
    the horizontal shift sx into a flat free-dim offset.
  - vertical 8-row box-sum: matmul with 0/1 A-matrices over partitions;
    horizontal box-sum folded into PSUM accumulation over xx offsets.
    Channel-group c4 = col-tile group (tile_position=(0,32*c4)) -> 4
    concurrent accumulation chains; N=384 contiguous rhs per matmul.
  - PSUM -> SBUF on Scalar (A padded to M=32 so all 128 rows are real),
    DMA out 40 canonical cells only; mirrors assembled host-side.
"""

import functools
import os
import sys

import numpy as np

sys.path.insert(0, "/opt/trn_rl_repo")

import concourse.bass as bass  # noqa: E402,F401
import concourse.bacc as bacc  # noqa: E402
import concourse.mybir as mybir  # noqa: E402
from concourse import bass_utils  # noqa: E402
from concourse.tile import TileContext  # noqa: E402

B, C, H, W = 8, 64, 96, 96
NH = NW = 23
NCORES = 8
P = 128
NB = 3            # beta blocks of 128 superrows
C4, C16 = 4, 16   # channel groups x channels-per-group
APD = 26          # padded a-dim: ap = a+1, zeros at ap=0,25
FR = APD * C16    # 416: elems per (phase r) block
FB = 4 * FR       # 1664: elems per beta block
FD = NB * FB      # 4992: total free dim

fp32 = mybir.dt.float32
fp16 = mybir.dt.float16

# (c4, beta) combos that hold rows of channel-group c4
CB_LIST = [(0, 0), (1, 0), (1, 1), (2, 1), (2, 2), (3, 2)]
# per-c4 beta list
C4_BETAS = {0: [0], 1: [0, 1], 2: [1, 2], 3: [2]}
# load pieces: (beta, p0, p1, c4, h0)
LOAD_PIECES = [
    (0, 0, 96, 0, 0), (0, 96, 128, 1, 0), (1, 0, 64, 1, 32),
    (1, 64, 128, 2, 0), (2, 0, 32, 2, 64), (2, 32, 128, 3, 0),
]

# shifts whose products run on GpSimd instead of Vector
GPSIMD_SHIFTS = frozenset({(1, 2), (2, -2), (2, 2), (3, -2), (3, 2), (4, 2)})


def _canonical_cells():
    """Map canonical shift (sy>=0, sx) -> list of output cells (dy,dx)."""
    cells = {}
    for dy in range(8):
        for dx in range(8):
            sy, sx = dy - 4, dx - 4
            key = (sy, sx) if (sy > 0 or (sy == 0 and sx >= 0)) else (-sy, -sx)
            cells.setdefault(key, []).append((dy, dx))
    assert len(cells) == 40
    return cells


CELLS = _canonical_cells()
ORDER = sorted(CELLS.keys(), key=lambda s: (s[0], abs(s[1])))


def _amat_np():
    """Merged vertical box-sum weights, 192 cols per sy:
      [ 0: 64)  beta=0 pair: cols j<32 -> (c4=0, win j), j>=32 -> (c4=1, j-32)
      [64:128)  beta=2 pair: cols j<32 -> (c4=2, win j), j>=32 -> (c4=3, j-32)
      [128:160) beta=1, c4=1 ; [160:192) beta=1, c4=2
    A[p, col] = 1 if superrow g=128*beta+p is in channel-group c4 and its
    h lies in vertical window i for shift sy."""
    a = np.zeros((P, 5 * 192), np.float16)
    blocks = [(0, 0, 0), (0, 1, 32), (2, 2, 64), (2, 3, 96),
              (1, 1, 128), (1, 2, 160)]
    for sy in range(5):
        for beta, c4, cb in blocks:
            base = sy * 192 + cb
            for p in range(P):
                g = 128 * beta + p
                if g // 96 != c4:
                    continue
                h = g % 96
                for i in range(NH):
                    if 0 <= h - 4 * i < 8 - sy:
                        a[p, base + i] = 1.0
    return a


def build_nc():
    nc = bacc.Bacc()
    x_dram = nc.dram_tensor("x", [C, H, W], fp32, kind="ExternalInput")
    amat_dram = nc.dram_tensor("amat", [P, 5 * 192], fp16,
                               kind="ExternalInput")
    out_dram = nc.dram_tensor("out", [40, P, NH * C16], fp32,
                              kind="ExternalOutput")

    with TileContext(nc) as tc:
        with (
            tc.tile_pool(name="const", bufs=1) as cpool,
            tc.tile_pool(name="stage", bufs=1) as spool,
            tc.tile_pool(name="xt", bufs=1) as tpool,
            tc.tile_pool(name="q", bufs=4) as qpool,
            tc.tile_pool(name="o", bufs=2) as opool,
            tc.tile_pool(name="ps", bufs=3, space="PSUM") as ppool,
        ):
            amat_t = cpool.tile([P, 5 * 192], fp16)
            nc.sync.dma_start(amat_t, amat_dram[:, :])

            # ---- load x into staging, layout [p, (beta, c16, w)] fp32
            stage = spool.tile([P, NB * C16 * W], fp32)
            for beta, p0, p1, c4, h0 in LOAD_PIECES:
                src = x_dram[16 * c4:16 * (c4 + 1), h0:h0 + (p1 - p0), :]
                nc.sync.dma_start(
                    stage[p0:p1, beta * C16 * W:(beta + 1) * C16 * W],
                    src.rearrange("c h w -> h c w"),
                )

            # ---- X0: phase-major packed fp16, zero-padded
            x0 = tpool.tile([P, FD], fp16)
            nc.gpsimd.memset(x0, 0.0)
            x0v = x0.rearrange("p (b r a c) -> p b r a c", b=NB, r=4, a=APD)
            stgv = stage.rearrange("p (b c w) -> p b w c", b=NB, c=C16)
            for beta in range(NB):
                for r in range(4):
                    nc.scalar.copy(
                        x0v[:, beta, r, 1:25, :],
                        stgv[:, beta, r::4, :],
                    )

            # ---- vertical-shift copies T_sy (superrow g -> g+sy)
            tt = {0: x0}
            for sy in range(1, 5):
                t = tpool.tile([P, FD], fp16, name=f"T{sy}")
                tt[sy] = t
                nc.gpsimd.memset(t[96:P, 2 * FB:3 * FB], 0.0)
                nc.sync.dma_start(t[0:P - sy, :], x0[sy:P, :])
                nc.sync.dma_start(t[P - sy:P, 0:2 * FB], x0[0:sy, FB:3 * FB])

            # ---- per-shift: product -> col-tiled matmuls -> copy -> DMA out
            for ks, (sy, sx) in enumerate(ORDER):
                eng = nc.gpsimd if (sy, sx) in GPSIMD_SHIFTS else nc.vector
                q = qpool.tile([P, FD], fp16, tag="q")
                q3 = q.rearrange("p (b f) -> p b f", b=NB)
                x03 = x0.rearrange("p (b f) -> p b f", b=NB)
                t3 = tt[sy].rearrange("p (b f) -> p b f", b=NB)

                def prod(o0, o1, i0):
                    eng.tensor_mul(
                        q3[:, :, o0:o1],
                        x03[:, :, o0:o1],
                        t3[:, :, i0:i0 + (o1 - o0)],
                    )

                # edge pad columns skipped by the sx=+-4 trims stay zero
                # from the previous tenant of the q buffer (first tenants
                # are the full-span sy=0 shifts), so no memsets needed.
                if sx == 4:  # lambda=1 for all r; trim OOB tail (ap=25, r=3)
                    prod(0, 4 * FR - C16, C16)
                elif sx >= 0:
                    prod(0, (4 - sx) * FR, sx * FR)
                    if sx > 0:
                        prod((4 - sx) * FR, 4 * FR, C16)
                elif sx > -4:
                    s = -sx
                    prod(0, s * FR, (4 - s) * FR - C16)
                    prod(s * FR, 4 * FR, 0)
                else:  # sx == -4: lambda=-1 for all r; trim OOB head (ap=0, r=0)
                    prod(C16, FR, 0)
                    prod(FR, 4 * FR, FR - C16)

                # matmuls: merged col-tiled chains. Per xx offset:
                #   beta=0 pair (c4=0+1) M=64 @ col 0, beta=2 pair (c4=2+3)
                #   M=64 @ col 64, then beta=1 singles (c4=1 @ col 32,
                #   c4=2 @ col 64) accumulating on top.
                xlist = list(range(max(0, -sx), 8 - max(0, sx)))
                pt = ppool.tile([P, 384], fp32, tag="ps")
                mm_list = [(0, 0, 0, 64), (64, 2, 64, 64),
                           (32, 1, 128, 32), (64, 1, 160, 32)]
                for xi, xx in enumerate(xlist):
                    rx, jjx = xx & 3, xx >> 2
                    rhs_off = rx * FR + (jjx + 1) * C16
                    for prow, bb, acol, m in mm_list:
                        nc.tensor.matmul(
                            pt[prow:prow + m, :],
                            amat_t[:, sy * 192 + acol:sy * 192 + acol + m],
                            q3[:, bb, rhs_off:rhs_off + 384],
                            start=(xi == 0 and m == 64),
                            stop=(xi == len(xlist) - 1),
                            tile_position=(0, prow),
                            skip_group_check=True,
                        )

                o_t = opool.tile([P, 384], fp32, tag="o")
                nc.scalar.copy(o_t, pt)
                dma_eng = nc.sync if ks % 2 == 0 else nc.scalar
                dma_eng.dma_start(out_dram[ks], o_t[:, 0:NH * C16])

    if not nc.is_finalized():
        nc.finalize()
    return nc


@functools.lru_cache(maxsize=1)
def _get_nc():
    return build_nc()


def _run(x, trace=False):
    amat = _amat_np()
    nc = _get_nc()
    in_maps = [
        {"x": np.ascontiguousarray(x[b]), "amat": amat} for b in range(NCORES)
    ]
    return bass_utils.run_bass_kernel_spmd(
        nc, in_maps, core_ids=list(range(NCORES)), trace=trace,
    )


def kernel(**inputs) -> np.ndarray:
    x = np.asarray(inputs["x"], dtype=np.float32)
    assert x.shape == (B, C, H, W)
    res = _run(x, trace=bool(int(os.environ.get("KERNEL_TRACE", "0"))))
    outs = np.stack([r["out"] for r in res.results])  # [B, 40, 128, (j c16)]
    blk = outs.reshape(B, 40, C4, 32, NH, C16)[:, :, :, :NH]
    blk = blk.transpose(0, 1, 2, 5, 3, 4).reshape(B, 40, C, NH, NH)
    full = np.empty((B, C, NH, NH, 8, 8), np.float32)
    for ks, key in enumerate(ORDER):
        for dy, dx in CELLS[key]:
            full[:, :, :, :, dy, dx] = blk[:, ks]
    return full


if __name__ == "__main__":
    rng = np.random.default_rng(0)
    x = rng.standard_normal((B, C, H, W), dtype=np.float32)
    y = kernel(x=x)
    print("out", y.shape, y.dtype, float(np.abs(y).max()))


# revision 15
# speedup vs baseline: 1.4599x; 1.0130x over previous
"""LocalAutoCorr2D Trainium2 kernel (v2: packed phase-major layout).

out[b,c,i,j,dy,dx] = sum_{y,x valid} x[b,c,4i+y,4j+x] * x[b,c,4i+y+sy,4j+x+sx]
with (sy,sx) = (dy-4, dx-4), 8x8 windows at stride 4 on a 96x96 image,
zero-padded at window boundaries.  Batch-sharded over 8 cores.

Layout: superimage rows g = 96*(c div 16) + h, g in [0,384); partition
p = g mod 128, free dim = (beta=g div 128, r=w mod 4, ap=w div 4 + 1,
c16=c mod 16) with zero pad columns at ap=0 and ap=25.  FD = 3*4*26*16
= 4992 fp16 elements per partition.

Per canonical shift (40 classes after out[s]==out[-s] symmetry):
  - product Q = X0 .* T_sy (T_sy = partition-shifted copy, sy rows up)
    as 2-3 contiguous-slice DVE/GpSimd ops: the phase-major layout turns
    the horizontal shift sx into a flat free-dim offset.
  - vertical 8-row box-sum: matmul with 0/1 A-matrices over partitions;
    horizontal box-sum folded into PSUM accumulation over xx offsets.
    Channel-group c4 = col-tile group (tile_position=(0,32*c4)) -> 4
    concurrent accumulation chains; N=384 contiguous rhs per matmul.
  - PSUM -> SBUF on Scalar (A padded to M=32 so all 128 rows are real),
    DMA out 40 canonical cells only; mirrors assembled host-side.
"""

import functools
import os
import sys

import numpy as np

sys.path.insert(0, "/opt/trn_rl_repo")

import concourse.bass as bass  # noqa: E402,F401
import concourse.bacc as bacc  # noqa: E402
import concourse.mybir as mybir  # noqa: E402
from concourse import bass_utils  # noqa: E402
from concourse.tile import TileContext  # noqa: E402

B, C, H, W = 8, 64, 96, 96
NH = NW = 23
NCORES = 8
P = 128
NB = 3            # beta blocks of 128 superrows
C4, C16 = 4, 16   # channel groups x channels-per-group
APD = 26          # padded a-dim: ap = a+1, zeros at ap=0,25
FR = APD * C16    # 416: elems per (phase r) block
FB = 4 * FR       # 1664: elems per beta block
FD = NB * FB      # 4992: total free dim

fp32 = mybir.dt.float32
fp16 = mybir.dt.float16

# (c4, beta) combos that hold rows of channel-group c4
CB_LIST = [(0, 0), (1, 0), (1, 1), (2, 1), (2, 2), (3, 2)]
# per-c4 beta list
C4_BETAS = {0: [0], 1: [0, 1], 2: [1, 2], 3: [2]}
# load pieces: (beta, p0, p1, c4, h0)
LOAD_PIECES = [
    (0, 0, 96, 0, 0), (0, 96, 128, 1, 0), (1, 0, 64, 1, 32),
    (1, 64, 128, 2, 0), (2, 0, 32, 2, 64), (2, 32, 128, 3, 0),
]

# shifts whose products run on GpSimd instead of Vector
GPSIMD_SHIFTS = frozenset({(1, 2), (2, -2), (2, 2), (3, -2), (3, 2), (4, 2)})


def _canonical_cells():
    """Map canonical shift (sy>=0, sx) -> list of output cells (dy,dx)."""
    cells = {}
    for dy in range(8):
        for dx in range(8):
            sy, sx = dy - 4, dx - 4
            key = (sy, sx) if (sy > 0 or (sy == 0 and sx >= 0)) else (-sy, -sx)
            cells.setdefault(key, []).append((dy, dx))
    assert len(cells) == 40
    return cells


CELLS = _canonical_cells()
ORDER = sorted(CELLS.keys(), key=lambda s: (s[0], abs(s[1])))


def _amat_np():
    """Merged vertical box-sum weights, 192 cols per sy:
      [ 0: 64)  beta=0 pair: cols j<32 -> (c4=0, win j), j>=32 -> (c4=1, j-32)
      [64:128)  beta=2 pair: cols j<32 -> (c4=2, win j), j>=32 -> (c4=3, j-32)
      [128:160) beta=1, c4=1 ; [160:192) beta=1, c4=2
    A[p, col] = 1 if superrow g=128*beta+p is in channel-group c4 and its
    h lies in vertical window i for shift sy."""
    a = np.zeros((P, 5 * 192), np.float16)
    blocks = [(0, 0, 0), (0, 1, 32), (2, 2, 64), (2, 3, 96),
              (1, 1, 128), (1, 2, 160)]
    for sy in range(5):
        for beta, c4, cb in blocks:
            base = sy * 192 + cb
            for p in range(P):
                g = 128 * beta + p
                if g // 96 != c4:
                    continue
                h = g % 96
                for i in range(NH):
                    if 0 <= h - 4 * i < 8 - sy:
                        a[p, base + i] = 1.0
    return a


def build_nc():
    nc = bacc.Bacc()
    x_dram = nc.dram_tensor("x", [C, H, W], fp32, kind="ExternalInput")
    amat_dram = nc.dram_tensor("amat", [P, 5 * 192], fp16,
                               kind="ExternalInput")
    out_dram = nc.dram_tensor("out", [40, P, NH * C16], fp32,
                              kind="ExternalOutput")

    with TileContext(nc) as tc:
        with (
            tc.tile_pool(name="const", bufs=1) as cpool,
            tc.tile_pool(name="stage", bufs=1) as spool,
            tc.tile_pool(name="xt", bufs=1) as tpool,
            tc.tile_pool(name="q", bufs=4) as qpool,
            tc.tile_pool(name="o", bufs=3) as opool,
            tc.tile_pool(name="ps", bufs=3, space="PSUM") as ppool,
        ):
            amat_t = cpool.tile([P, 5 * 192], fp16)
            nc.sync.dma_start(amat_t, amat_dram[:, :])

            # ---- load x into staging, layout [p, (beta, c16, w)] fp32
            stage = spool.tile([P, NB * C16 * W], fp32)
            for beta, p0, p1, c4, h0 in LOAD_PIECES:
                src = x_dram[16 * c4:16 * (c4 + 1), h0:h0 + (p1 - p0), :]
                nc.sync.dma_start(
                    stage[p0:p1, beta * C16 * W:(beta + 1) * C16 * W],
                    src.rearrange("c h w -> h c w"),
                )

            # ---- X0: phase-major packed fp16, zero-padded
            x0 = tpool.tile([P, FD], fp16)
            x0v = x0.rearrange("p (b r a c) -> p b r a c", b=NB, r=4, a=APD)
            x0q = x0.rearrange("p (q a c) -> p q a c", q=NB * 4, a=APD)
            nc.gpsimd.memset(x0q[:, :, 0, :], 0.0)
            nc.gpsimd.memset(x0q[:, :, 25, :], 0.0)
            stgv = stage.rearrange("p (b c w) -> p b w c", b=NB, c=C16)
            for beta in range(NB):
                for r in range(4):
                    nc.scalar.copy(
                        x0v[:, beta, r, 1:25, :],
                        stgv[:, beta, r::4, :],
                    )

            # ---- vertical-shift copies T_sy (superrow g -> g+sy), spread
            # across engine queues so the transfers run concurrently
            tt = {0: x0}
            engs = [nc.sync, nc.scalar, nc.gpsimd, nc.sync]
            for sy in range(1, 5):
                t = tpool.tile([P, FD], fp16, name=f"T{sy}")
                tt[sy] = t
                nc.gpsimd.memset(t[96:P, 2 * FB:3 * FB], 0.0)
                for half in range(2):
                    engs[(2 * (sy - 1) + half) % 4].dma_start(
                        t[0:P - sy, half * 2496:(half + 1) * 2496],
                        x0[sy:P, half * 2496:(half + 1) * 2496],
                    )
                engs[sy % 4].dma_start(
                    t[P - sy:P, 0:2 * FB], x0[0:sy, FB:3 * FB]
                )

            # ---- per-shift: product -> col-tiled matmuls -> copy -> DMA out
            for ks, (sy, sx) in enumerate(ORDER):
                eng = nc.gpsimd if (sy, sx) in GPSIMD_SHIFTS else nc.vector
                q = qpool.tile([P, FD], fp16, tag="q")
                q3 = q.rearrange("p (b f) -> p b f", b=NB)
                x03 = x0.rearrange("p (b f) -> p b f", b=NB)
                t3 = tt[sy].rearrange("p (b f) -> p b f", b=NB)

                def prod(o0, o1, i0):
                    eng.tensor_mul(
                        q3[:, :, o0:o1],
                        x03[:, :, o0:o1],
                        t3[:, :, i0:i0 + (o1 - o0)],
                    )

                # edge pad columns skipped by the sx=+-4 trims stay zero
                # from the previous tenant of the q buffer (first tenants
                # are the full-span sy=0 shifts), so no memsets needed.
                if sx == 4:  # lambda=1 for all r; trim OOB tail (ap=25, r=3)
                    prod(0, 4 * FR - C16, C16)
                elif sx >= 0:
                    prod(0, (4 - sx) * FR, sx * FR)
                    if sx > 0:
                        prod((4 - sx) * FR, 4 * FR, C16)
                elif sx > -4:
                    s = -sx
                    prod(0, s * FR, (4 - s) * FR - C16)
                    prod(s * FR, 4 * FR, 0)
                else:  # sx == -4: lambda=-1 for all r; trim OOB head (ap=0, r=0)
                    prod(C16, FR, 0)
                    prod(FR, 4 * FR, FR - C16)

                # matmuls: merged col-tiled chains. Per xx offset:
                #   beta=0 pair (c4=0+1) M=64 @ col 0, beta=2 pair (c4=2+3)
                #   M=64 @ col 64, then beta=1 singles (c4=1 @ col 32,
                #   c4=2 @ col 64) accumulating on top.
                xlist = list(range(max(0, -sx), 8 - max(0, sx)))
                pt = ppool.tile([P, 384], fp32, tag="ps")
                mm_list = [(0, 0, 0, 64), (64, 2, 64, 64),
                           (32, 1, 128, 32), (64, 1, 160, 32)]
                for xi, xx in enumerate(xlist):
                    rx, jjx = xx & 3, xx >> 2
                    rhs_off = rx * FR + (jjx + 1) * C16
                    for prow, bb, acol, m in mm_list:
                        nc.tensor.matmul(
                            pt[prow:prow + m, :],
                            amat_t[:, sy * 192 + acol:sy * 192 + acol + m],
                            q3[:, bb, rhs_off:rhs_off + 384],
                            start=(xi == 0 and m == 64),
                            stop=(xi == len(xlist) - 1),
                            tile_position=(0, prow),
                            skip_group_check=True,
                        )

                o_t = opool.tile([P, 384], fp32, tag="o")
                nc.scalar.copy(o_t, pt)
                dma_eng = nc.sync if ks % 2 == 0 else nc.scalar
                dma_eng.dma_start(out_dram[ks], o_t[:, 0:NH * C16])

    if not nc.is_finalized():
        nc.finalize()
    return nc


@functools.lru_cache(maxsize=1)
def _get_nc():
    return build_nc()


def _run(x, trace=False):
    amat = _amat_np()
    nc = _get_nc()
    in_maps = [
        {"x": np.ascontiguousarray(x[b]), "amat": amat} for b in range(NCORES)
    ]
    return bass_utils.run_bass_kernel_spmd(
        nc, in_maps, core_ids=list(range(NCORES)), trace=trace,
    )


def kernel(**inputs) -> np.ndarray:
    x = np.asarray(inputs["x"], dtype=np.float32)
    assert x.shape == (B, C, H, W)
    res = _run(x, trace=bool(int(os.environ.get("KERNEL_TRACE", "0"))))
    outs = np.stack([r["out"] for r in res.results])  # [B, 40, 128, (j c16)]
    blk = outs.reshape(B, 40, C4, 32, NH, C16)[:, :, :, :NH]
    blk = blk.transpose(0, 1, 2, 5, 3, 4).reshape(B, 40, C, NH, NH)
    full = np.empty((B, C, NH, NH, 8, 8), np.float32)
    for ks, key in enumerate(ORDER):
        for dy, dx in CELLS[key]:
            full[:, :, :, :, dy, dx] = blk[:, ks]
    return full


if __name__ == "__main__":
    rng = np.random.default_rng(0)
    x = rng.standard_normal((B, C, H, W), dtype=np.float32)
    y = kernel(x=x)
    print("out", y.shape, y.dtype, float(np.abs(y).max()))


# revision 16
# speedup vs baseline: 1.7262x; 1.1825x over previous
"""LocalAutoCorr2D Trainium2 kernel (v2: packed phase-major layout).

out[b,c,i,j,dy,dx] = sum_{y,x valid} x[b,c,4i+y,4j+x] * x[b,c,4i+y+sy,4j+x+sx]
with (sy,sx) = (dy-4, dx-4), 8x8 windows at stride 4 on a 96x96 image,
zero-padded at window boundaries.  Batch-sharded over 8 cores.

Layout: superimage rows g = 96*(c div 16) + h, g in [0,384); partition
p = g mod 128, free dim = (beta=g div 128, r=w mod 4, ap=w div 4 + 1,
c16=c mod 16) with zero pad columns at ap=0 and ap=25.  FD = 3*4*26*16
= 4992 fp16 elements per partition.

Per canonical shift (40 classes after out[s]==out[-s] symmetry):
  - product Q = X0 .* T_sy (T_sy = partition-shifted copy, sy rows up)
    as 2-3 contiguous-slice DVE/GpSimd ops: the phase-major layout turns
    the horizontal shift sx into a flat free-dim offset.
  - vertical 8-row box-sum: matmul with 0/1 A-matrices over partitions;
    horizontal box-sum folded into PSUM accumulation over xx offsets.
    Channel-group c4 = col-tile group (tile_position=(0,32*c4)) -> 4
    concurrent accumulation chains; N=384 contiguous rhs per matmul.
  - PSUM -> SBUF on Scalar (A padded to M=32 so all 128 rows are real),
    DMA out 40 canonical cells only; mirrors assembled host-side.
"""

import functools
import os
import sys

import numpy as np

sys.path.insert(0, "/opt/trn_rl_repo")

import concourse.bass as bass  # noqa: E402,F401
import concourse.bacc as bacc  # noqa: E402
import concourse.mybir as mybir  # noqa: E402
from concourse import bass_utils  # noqa: E402
from concourse.tile import TileContext  # noqa: E402

B, C, H, W = 8, 64, 96, 96
NH = NW = 23
NCORES = 8
P = 128
NB = 3            # beta blocks of 128 superrows
C4, C16 = 4, 16   # channel groups x channels-per-group
APD = 26          # padded a-dim: ap = a+1, zeros at ap=0,25
FR = APD * C16    # 416: elems per (phase r) block
FB = 4 * FR       # 1664: elems per beta block
FD = NB * FB      # 4992: total free dim

fp32 = mybir.dt.float32
fp16 = mybir.dt.float16

# (c4, beta) combos that hold rows of channel-group c4
CB_LIST = [(0, 0), (1, 0), (1, 1), (2, 1), (2, 2), (3, 2)]
# per-c4 beta list
C4_BETAS = {0: [0], 1: [0, 1], 2: [1, 2], 3: [2]}
# load pieces: (beta, p0, p1, c4, h0)
LOAD_PIECES = [
    (0, 0, 96, 0, 0), (0, 96, 128, 1, 0), (1, 0, 64, 1, 32),
    (1, 64, 128, 2, 0), (2, 0, 32, 2, 64), (2, 32, 128, 3, 0),
]

# shifts whose products run on GpSimd instead of Vector
GPSIMD_SHIFTS = frozenset({(1, 2), (2, -2), (2, 2), (3, -2), (3, 2), (4, 2)})


def _canonical_cells():
    """Map canonical shift (sy>=0, sx) -> list of output cells (dy,dx)."""
    cells = {}
    for dy in range(8):
        for dx in range(8):
            sy, sx = dy - 4, dx - 4
            key = (sy, sx) if (sy > 0 or (sy == 0 and sx >= 0)) else (-sy, -sx)
            cells.setdefault(key, []).append((dy, dx))
    assert len(cells) == 40
    return cells


CELLS = _canonical_cells()
ORDER = sorted(CELLS.keys(), key=lambda s: (s[0], abs(s[1])))


def _amat_np():
    """Merged vertical box-sum weights, 192 cols per sy:
      [ 0: 64)  beta=0 pair: cols j<32 -> (c4=0, win j), j>=32 -> (c4=1, j-32)
      [64:128)  beta=2 pair: cols j<32 -> (c4=2, win j), j>=32 -> (c4=3, j-32)
      [128:160) beta=1, c4=1 ; [160:192) beta=1, c4=2
    A[p, col] = 1 if superrow g=128*beta+p is in channel-group c4 and its
    h lies in vertical window i for shift sy."""
    a = np.zeros((P, 5 * 192), np.float16)
    blocks = [(0, 0, 0), (0, 1, 32), (2, 2, 64), (2, 3, 96),
              (1, 1, 128), (1, 2, 160)]
    for sy in range(5):
        for beta, c4, cb in blocks:
            base = sy * 192 + cb
            for p in range(P):
                g = 128 * beta + p
                if g // 96 != c4:
                    continue
                h = g % 96
                for i in range(NH):
                    if 0 <= h - 4 * i < 8 - sy:
                        a[p, base + i] = 1.0
    return a


def build_nc():
    nc = bacc.Bacc()
    x_dram = nc.dram_tensor("x", [C, H, W], fp32, kind="ExternalInput")
    amat_dram = nc.dram_tensor("amat", [P, 5 * 192], fp16,
                               kind="ExternalInput")
    out_dram = nc.dram_tensor("out", [40, P, NH * C16], fp32,
                              kind="ExternalOutput")

    with TileContext(nc) as tc:
        with (
            tc.tile_pool(name="const", bufs=1) as cpool,
            tc.tile_pool(name="stage", bufs=1) as spool,
            tc.tile_pool(name="xt", bufs=1) as tpool,
            tc.tile_pool(name="q", bufs=4) as qpool,
            tc.tile_pool(name="o", bufs=3) as opool,
            tc.tile_pool(name="ps", bufs=3, space="PSUM") as ppool,
        ):
            amat_t = cpool.tile([P, 5 * 192], fp16)
            nc.sync.dma_start(amat_t, amat_dram[:, :])

            # ---- load x into staging, layout [p, (beta, c16, w)] fp32
            stage = spool.tile([P, NB * C16 * W], fp32)
            for beta, p0, p1, c4, h0 in LOAD_PIECES:
                src = x_dram[16 * c4:16 * (c4 + 1), h0:h0 + (p1 - p0), :]
                nc.sync.dma_start(
                    stage[p0:p1, beta * C16 * W:(beta + 1) * C16 * W],
                    src.rearrange("c h w -> h c w"),
                )

            # ---- X0: phase-major packed fp16, zero-padded
            x0 = tpool.tile([P, FD], fp16)
            x0v = x0.rearrange("p (b r a c) -> p b r a c", b=NB, r=4, a=APD)
            x0q = x0.rearrange("p (q a c) -> p q a c", q=NB * 4, a=APD)
            nc.gpsimd.memset(x0q[:, :, 0, :], 0.0)
            nc.gpsimd.memset(x0q[:, :, 25, :], 0.0)
            stgv = stage.rearrange("p (b c w) -> p b w c", b=NB, c=C16)
            for beta in range(NB):
                for r in range(4):
                    nc.scalar.copy(
                        x0v[:, beta, r, 1:25, :],
                        stgv[:, beta, r::4, :],
                    )

            # ---- vertical-shift copies T_sy (superrow g -> g+sy), spread
            # across engine queues so the transfers run concurrently
            tt = {0: x0}
            engs = [nc.sync, nc.scalar, nc.gpsimd, nc.sync]
            for sy in range(1, 5):
                t = tpool.tile([P, FD], fp16, name=f"T{sy}")
                tt[sy] = t
                nc.gpsimd.memset(t[96:P, 2 * FB:3 * FB], 0.0)
                for half in range(2):
                    engs[(2 * (sy - 1) + half) % 4].dma_start(
                        t[0:P - sy, half * 2496:(half + 1) * 2496],
                        x0[sy:P, half * 2496:(half + 1) * 2496],
                    )
                engs[sy % 4].dma_start(
                    t[P - sy:P, 0:2 * FB], x0[0:sy, FB:3 * FB]
                )

            # ---- per-shift: product -> col-tiled matmuls -> copy -> DMA out
            for ks, (sy, sx) in enumerate(ORDER):
                eng = nc.gpsimd if (sy, sx) in GPSIMD_SHIFTS else nc.vector
                q = qpool.tile([P, FD], fp16, tag="q")
                q3 = q.rearrange("p (b f) -> p b f", b=NB)
                x03 = x0.rearrange("p (b f) -> p b f", b=NB)
                t3 = tt[sy].rearrange("p (b f) -> p b f", b=NB)

                def prod(o0, o1, i0):
                    eng.tensor_mul(
                        q3[:, :, o0:o1],
                        x03[:, :, o0:o1],
                        t3[:, :, i0:i0 + (o1 - o0)],
                    )

                # edge pad columns skipped by the sx=+-4 trims stay zero
                # from the previous tenant of the q buffer (first tenants
                # are the full-span sy=0 shifts), so no memsets needed.
                if sx == 4:  # lambda=1 for all r; trim OOB tail (ap=25, r=3)
                    prod(0, 4 * FR - C16, C16)
                elif sx >= 0:
                    prod(0, (4 - sx) * FR, sx * FR)
                    if sx > 0:
                        prod((4 - sx) * FR, 4 * FR, C16)
                elif sx > -4:
                    s = -sx
                    prod(0, s * FR, (4 - s) * FR - C16)
                    prod(s * FR, 4 * FR, 0)
                else:  # sx == -4: lambda=-1 for all r; trim OOB head (ap=0, r=0)
                    prod(C16, FR, 0)
                    prod(FR, 4 * FR, FR - C16)

                # matmuls: 4 independent accumulation chains, one per c4
                # (PSUM rows [32*c4, +32), col-tile position 32*c4).
                # Chains never share PSUM rows, so the PE overlaps them;
                # emission order keeps adjacent mms on distinct positions.
                # (c4, beta, amat col): chain c4=1 has beta 0+1, c4=2 has 1+2
                mm_seq = [(0, 0, 0), (1, 0, 32), (2, 1, 160),
                          (3, 2, 96), (1, 1, 128), (2, 2, 64)]
                chain_first = {0: 0, 1: 1, 2: 2, 3: 3}
                chain_last = {0: 0, 1: 4, 2: 5, 3: 3}
                xlist = list(range(max(0, -sx), 8 - max(0, sx)))
                pt = ppool.tile([P, 384], fp32, tag="ps")
                for xi, xx in enumerate(xlist):
                    rx, jjx = xx & 3, xx >> 2
                    rhs_off = rx * FR + (jjx + 1) * C16
                    for mi, (c4, bb, acol) in enumerate(mm_seq):
                        nc.tensor.matmul(
                            pt[32 * c4:32 * c4 + 32, :],
                            amat_t[:, sy * 192 + acol:sy * 192 + acol + 32],
                            q3[:, bb, rhs_off:rhs_off + 384],
                            start=(xi == 0 and chain_first[c4] == mi),
                            stop=(xi == len(xlist) - 1
                                  and chain_last[c4] == mi),
                            tile_position=(0, 32 * c4),
                            skip_group_check=True,
                        )

                o_t = opool.tile([P, 384], fp32, tag="o")
                nc.scalar.copy(o_t, pt)
                dma_eng = nc.sync if ks % 2 == 0 else nc.scalar
                dma_eng.dma_start(out_dram[ks], o_t[:, 0:NH * C16])

    if not nc.is_finalized():
        nc.finalize()
    return nc


@functools.lru_cache(maxsize=1)
def _get_nc():
    return build_nc()


def _run(x, trace=False):
    amat = _amat_np()
    nc = _get_nc()
    in_maps = [
        {"x": np.ascontiguousarray(x[b]), "amat": amat} for b in range(NCORES)
    ]
    return bass_utils.run_bass_kernel_spmd(
        nc, in_maps, core_ids=list(range(NCORES)), trace=trace,
    )


def kernel(**inputs) -> np.ndarray:
    x = np.asarray(inputs["x"], dtype=np.float32)
    assert x.shape == (B, C, H, W)
    res = _run(x, trace=bool(int(os.environ.get("KERNEL_TRACE", "0"))))
    outs = np.stack([r["out"] for r in res.results])  # [B, 40, 128, (j c16)]
    blk = outs.reshape(B, 40, C4, 32, NH, C16)[:, :, :, :NH]
    blk = blk.transpose(0, 1, 2, 5, 3, 4).reshape(B, 40, C, NH, NH)
    full = np.empty((B, C, NH, NH, 8, 8), np.float32)
    for ks, key in enumerate(ORDER):
        for dy, dx in CELLS[key]:
            full[:, :, :, :, dy, dx] = blk[:, ks]
    return full


if __name__ == "__main__":
    rng = np.random.default_rng(0)
    x = rng.standard_normal((B, C, H, W), dtype=np.float32)
    y = kernel(x=x)
    print("out", y.shape, y.dtype, float(np.abs(y).max()))


# revision 18
# speedup vs baseline: 2.5061x; 1.4518x over previous
"""LocalAutoCorr2D Trainium2 kernel (v2: packed phase-major layout).

out[b,c,i,j,dy,dx] = sum_{y,x valid} x[b,c,4i+y,4j+x] * x[b,c,4i+y+sy,4j+x+sx]
with (sy,sx) = (dy-4, dx-4), 8x8 windows at stride 4 on a 96x96 image,
zero-padded at window boundaries.  Batch-sharded over 8 cores.

Layout: superimage rows g = 96*(c div 16) + h, g in [0,384); partition
p = g mod 128, free dim = (beta=g div 128, r=w mod 4, ap=w div 4 + 1,
c16=c mod 16) with zero pad columns at ap=0 and ap=25.  FD = 3*4*26*16
= 4992 fp16 elements per partition.

Per canonical shift (40 classes after out[s]==out[-s] symmetry):
  - product Q = X0 .* T_sy (T_sy = partition-shifted copy, sy rows up)
    as 2-3 contiguous-slice DVE/GpSimd ops: the phase-major layout turns
    the horizontal shift sx into a flat free-dim offset.
  - vertical 8-row box-sum: matmul with 0/1 A-matrices over partitions;
    horizontal box-sum folded into PSUM accumulation over xx offsets.
    Channel-group c4 = col-tile group (tile_position=(0,32*c4)) -> 4
    concurrent accumulation chains; N=384 contiguous rhs per matmul.
  - PSUM -> SBUF on Scalar (A padded to M=32 so all 128 rows are real),
    DMA out 40 canonical cells only; mirrors assembled host-side.
"""

import functools
import os
import sys

import numpy as np

sys.path.insert(0, "/opt/trn_rl_repo")

import concourse.bass as bass  # noqa: E402,F401
import concourse.bacc as bacc  # noqa: E402
import concourse.mybir as mybir  # noqa: E402
from concourse import bass_utils  # noqa: E402
from concourse.tile import TileContext  # noqa: E402

B, C, H, W = 8, 64, 96, 96
NH = NW = 23
NCORES = 8
P = 128
NB = 3            # beta blocks of 128 superrows
C4, C16 = 4, 16   # channel groups x channels-per-group
APD = 26          # padded a-dim: ap = a+1, zeros at ap=0,25
FR = APD * C16    # 416: elems per (phase r) block
FB = 4 * FR       # 1664: elems per beta block
FD = NB * FB      # 4992: total free dim

fp32 = mybir.dt.float32
fp16 = mybir.dt.float16

# (c4, beta) combos that hold rows of channel-group c4
CB_LIST = [(0, 0), (1, 0), (1, 1), (2, 1), (2, 2), (3, 2)]
# per-c4 beta list
C4_BETAS = {0: [0], 1: [0, 1], 2: [1, 2], 3: [2]}
# load pieces: (beta, p0, p1, c4, h0)
LOAD_PIECES = [
    (0, 0, 96, 0, 0), (0, 96, 128, 1, 0), (1, 0, 64, 1, 32),
    (1, 64, 128, 2, 0), (2, 0, 32, 2, 64), (2, 32, 128, 3, 0),
]

# shifts whose products run on GpSimd instead of Vector (GpSimd
# tensor_tensor measured ~4x slower than DVE and contends badly; keep empty)
GPSIMD_SHIFTS = frozenset()


def _canonical_cells():
    """Map canonical shift (sy>=0, sx) -> list of output cells (dy,dx)."""
    cells = {}
    for dy in range(8):
        for dx in range(8):
            sy, sx = dy - 4, dx - 4
            key = (sy, sx) if (sy > 0 or (sy == 0 and sx >= 0)) else (-sy, -sx)
            cells.setdefault(key, []).append((dy, dx))
    assert len(cells) == 40
    return cells


CELLS = _canonical_cells()
ORDER = sorted(CELLS.keys(), key=lambda s: (s[0], abs(s[1])))


def _amat_np():
    """Merged vertical box-sum weights, 192 cols per sy:
      [ 0: 64)  beta=0 pair: cols j<32 -> (c4=0, win j), j>=32 -> (c4=1, j-32)
      [64:128)  beta=2 pair: cols j<32 -> (c4=2, win j), j>=32 -> (c4=3, j-32)
      [128:160) beta=1, c4=1 ; [160:192) beta=1, c4=2
    A[p, col] = 1 if superrow g=128*beta+p is in channel-group c4 and its
    h lies in vertical window i for shift sy."""
    a = np.zeros((P, 5 * 192), np.float16)
    blocks = [(0, 0, 0), (0, 1, 32), (2, 2, 64), (2, 3, 96),
              (1, 1, 128), (1, 2, 160)]
    for sy in range(5):
        for beta, c4, cb in blocks:
            base = sy * 192 + cb
            for p in range(P):
                g = 128 * beta + p
                if g // 96 != c4:
                    continue
                h = g % 96
                for i in range(NH):
                    if 0 <= h - 4 * i < 8 - sy:
                        a[p, base + i] = 1.0
    return a


def build_nc():
    nc = bacc.Bacc()
    x_dram = nc.dram_tensor("x", [C, H, W], fp32, kind="ExternalInput")
    amat_dram = nc.dram_tensor("amat", [P, 5 * 192], fp16,
                               kind="ExternalInput")
    out_dram = nc.dram_tensor("out", [40, P, NH * C16], fp32,
                              kind="ExternalOutput")

    with TileContext(nc) as tc:
        with (
            tc.tile_pool(name="const", bufs=1) as cpool,
            tc.tile_pool(name="stage", bufs=1) as spool,
            tc.tile_pool(name="xt", bufs=1) as tpool,
            tc.tile_pool(name="q", bufs=4) as qpool,
            tc.tile_pool(name="o", bufs=3) as opool,
            tc.tile_pool(name="ps", bufs=3, space="PSUM") as ppool,
        ):
            amat_t = cpool.tile([P, 5 * 192], fp16)
            nc.sync.dma_start(amat_t, amat_dram[:, :])

            # ---- load x into staging, layout [p, (beta, c16, w)] fp32
            stage = spool.tile([P, NB * C16 * W], fp32)
            for beta, p0, p1, c4, h0 in LOAD_PIECES:
                src = x_dram[16 * c4:16 * (c4 + 1), h0:h0 + (p1 - p0), :]
                nc.sync.dma_start(
                    stage[p0:p1, beta * C16 * W:(beta + 1) * C16 * W],
                    src.rearrange("c h w -> h c w"),
                )

            # ---- X0: phase-major packed fp16, zero-padded
            x0 = tpool.tile([P, FD], fp16)
            x0v = x0.rearrange("p (b r a c) -> p b r a c", b=NB, r=4, a=APD)
            x0q = x0.rearrange("p (q a c) -> p q a c", q=NB * 4, a=APD)
            nc.gpsimd.memset(x0q[:, :, 0, :], 0.0)
            nc.gpsimd.memset(x0q[:, :, 25, :], 0.0)
            stgv = stage.rearrange("p (b c w) -> p b w c", b=NB, c=C16)
            for beta in range(NB):
                for r in range(4):
                    nc.scalar.copy(
                        x0v[:, beta, r, 1:25, :],
                        stgv[:, beta, r::4, :],
                    )

            # ---- vertical-shift copies T_sy (superrow g -> g+sy).
            # SBUF->SBUF DMA issue is fast only on gpsimd (async); on
            # sync/scalar the instruction blocks for the whole transfer.
            tt = {0: x0}
            for sy in range(1, 5):
                t = tpool.tile([P, FD], fp16, name=f"T{sy}")
                tt[sy] = t
                nc.gpsimd.memset(t[96:P, 2 * FB:3 * FB], 0.0)
                for half in range(2):
                    nc.gpsimd.dma_start(
                        t[0:P - sy, half * 2496:(half + 1) * 2496],
                        x0[sy:P, half * 2496:(half + 1) * 2496],
                    )
                nc.gpsimd.dma_start(
                    t[P - sy:P, 0:2 * FB], x0[0:sy, FB:3 * FB]
                )

            # ---- per-shift: product -> col-tiled matmuls -> copy -> DMA out
            for ks, (sy, sx) in enumerate(ORDER):
                eng = nc.gpsimd if (sy, sx) in GPSIMD_SHIFTS else nc.vector
                q = qpool.tile([P, FD], fp16, tag="q")
                q3 = q.rearrange("p (b f) -> p b f", b=NB)
                x03 = x0.rearrange("p (b f) -> p b f", b=NB)
                t3 = tt[sy].rearrange("p (b f) -> p b f", b=NB)

                def prod(o0, o1, i0):
                    eng.tensor_mul(
                        q3[:, :, o0:o1],
                        x03[:, :, o0:o1],
                        t3[:, :, i0:i0 + (o1 - o0)],
                    )

                # edge pad columns skipped by the sx=+-4 trims stay zero
                # from the previous tenant of the q buffer (first tenants
                # are the full-span sy=0 shifts), so no memsets needed.
                if sx == 4:  # lambda=1 for all r; trim OOB tail (ap=25, r=3)
                    prod(0, 4 * FR - C16, C16)
                elif sx >= 0:
                    prod(0, (4 - sx) * FR, sx * FR)
                    if sx > 0:
                        prod((4 - sx) * FR, 4 * FR, C16)
                elif sx > -4:
                    s = -sx
                    prod(0, s * FR, (4 - s) * FR - C16)
                    prod(s * FR, 4 * FR, 0)
                else:  # sx == -4: lambda=-1 for all r; trim OOB head (ap=0, r=0)
                    prod(C16, FR, 0)
                    prod(FR, 4 * FR, FR - C16)

                # matmuls: 4 independent accumulation chains, one per c4
                # (PSUM rows [32*c4, +32), col-tile position 32*c4).
                # Chains never share PSUM rows, so the PE overlaps them;
                # emission order keeps adjacent mms on distinct positions.
                # (c4, beta, amat col): chain c4=1 has beta 0+1, c4=2 has 1+2
                mm_seq = [(0, 0, 0), (1, 0, 32), (2, 1, 160),
                          (3, 2, 96), (1, 1, 128), (2, 2, 64)]
                chain_first = {0: 0, 1: 1, 2: 2, 3: 3}
                chain_last = {0: 0, 1: 4, 2: 5, 3: 3}
                xlist = list(range(max(0, -sx), 8 - max(0, sx)))
                pt = ppool.tile([P, 384], fp32, tag="ps")
                for xi, xx in enumerate(xlist):
                    rx, jjx = xx & 3, xx >> 2
                    rhs_off = rx * FR + (jjx + 1) * C16
                    for mi, (c4, bb, acol) in enumerate(mm_seq):
                        nc.tensor.matmul(
                            pt[32 * c4:32 * c4 + 32, :],
                            amat_t[:, sy * 192 + acol:sy * 192 + acol + 32],
                            q3[:, bb, rhs_off:rhs_off + 384],
                            start=(xi == 0 and chain_first[c4] == mi),
                            stop=(xi == len(xlist) - 1
                                  and chain_last[c4] == mi),
                            tile_position=(0, 32 * c4),
                            skip_group_check=True,
                        )

                o_t = opool.tile([P, 384], fp32, tag="o")
                nc.scalar.copy(o_t, pt)
                dma_eng = nc.sync if ks % 2 == 0 else nc.scalar
                dma_eng.dma_start(out_dram[ks], o_t[:, 0:NH * C16])

    if not nc.is_finalized():
        nc.finalize()
    return nc


@functools.lru_cache(maxsize=1)
def _get_nc():
    return build_nc()


def _run(x, trace=False):
    amat = _amat_np()
    nc = _get_nc()
    in_maps = [
        {"x": np.ascontiguousarray(x[b]), "amat": amat} for b in range(NCORES)
    ]
    return bass_utils.run_bass_kernel_spmd(
        nc, in_maps, core_ids=list(range(NCORES)), trace=trace,
    )


def kernel(**inputs) -> np.ndarray:
    x = np.asarray(inputs["x"], dtype=np.float32)
    assert x.shape == (B, C, H, W)
    res = _run(x, trace=bool(int(os.environ.get("KERNEL_TRACE", "0"))))
    outs = np.stack([r["out"] for r in res.results])  # [B, 40, 128, (j c16)]
    blk = outs.reshape(B, 40, C4, 32, NH, C16)[:, :, :, :NH]
    blk = blk.transpose(0, 1, 2, 5, 3, 4).reshape(B, 40, C, NH, NH)
    full = np.empty((B, C, NH, NH, 8, 8), np.float32)
    for ks, key in enumerate(ORDER):
        for dy, dx in CELLS[key]:
            full[:, :, :, :, dy, dx] = blk[:, ks]
    return full


if __name__ == "__main__":
    rng = np.random.default_rng(0)
    x = rng.standard_normal((B, C, H, W), dtype=np.float32)
    y = kernel(x=x)
    print("out", y.shape, y.dtype, float(np.abs(y).max()))


# revision 22
# speedup vs baseline: 2.5183x; 1.0049x over previous
"""LocalAutoCorr2D Trainium2 kernel (v2: packed phase-major layout).

out[b,c,i,j,dy,dx] = sum_{y,x valid} x[b,c,4i+y,4j+x] * x[b,c,4i+y+sy,4j+x+sx]
with (sy,sx) = (dy-4, dx-4), 8x8 windows at stride 4 on a 96x96 image,
zero-padded at window boundaries.  Batch-sharded over 8 cores.

Layout: superimage rows g = 96*(c div 16) + h, g in [0,384); partition
p = g mod 128, free dim = (beta=g div 128, r=w mod 4, ap=w div 4 + 1,
c16=c mod 16) with zero pad columns at ap=0 and ap=25.  FD = 3*4*26*16
= 4992 fp16 elements per partition.

Per canonical shift (40 classes after out[s]==out[-s] symmetry):
  - product Q = X0 .* T_sy (T_sy = partition-shifted copy, sy rows up)
    as 2-3 contiguous-slice DVE/GpSimd ops: the phase-major layout turns
    the horizontal shift sx into a flat free-dim offset.
  - vertical 8-row box-sum: matmul with 0/1 A-matrices over partitions;
    horizontal box-sum folded into PSUM accumulation over xx offsets.
    Channel-group c4 = col-tile group (tile_position=(0,32*c4)) -> 4
    concurrent accumulation chains; N=384 contiguous rhs per matmul.
  - PSUM -> SBUF on Scalar (A padded to M=32 so all 128 rows are real),
    DMA out 40 canonical cells only; mirrors assembled host-side.
"""

import functools
import os
import sys

import numpy as np

sys.path.insert(0, "/opt/trn_rl_repo")

import concourse.bass as bass  # noqa: E402,F401
import concourse.bacc as bacc  # noqa: E402
import concourse.mybir as mybir  # noqa: E402
from concourse import bass_utils  # noqa: E402
from concourse.tile import TileContext  # noqa: E402

B, C, H, W = 8, 64, 96, 96
NH = NW = 23
NCORES = 8
P = 128
NB = 3            # beta blocks of 128 superrows
C4, C16 = 4, 16   # channel groups x channels-per-group
APD = 26          # padded a-dim: ap = a+1, zeros at ap=0,25
FR = APD * C16    # 416: elems per (phase r) block
FB = 4 * FR       # 1664: elems per beta block
FD = NB * FB      # 4992: total free dim

fp32 = mybir.dt.float32
fp16 = mybir.dt.float16

# (c4, beta) combos that hold rows of channel-group c4
CB_LIST = [(0, 0), (1, 0), (1, 1), (2, 1), (2, 2), (3, 2)]
# per-c4 beta list
C4_BETAS = {0: [0], 1: [0, 1], 2: [1, 2], 3: [2]}
# load pieces: (beta, p0, p1, c4, h0)
LOAD_PIECES = [
    (0, 0, 96, 0, 0), (0, 96, 128, 1, 0), (1, 0, 64, 1, 32),
    (1, 64, 128, 2, 0), (2, 0, 32, 2, 64), (2, 32, 128, 3, 0),
]

# shifts whose products run on GpSimd instead of Vector (GpSimd
# tensor_tensor measured ~4x slower than DVE and contends badly; keep empty)
GPSIMD_SHIFTS = frozenset()


def _canonical_cells():
    """Map canonical shift (sy>=0, sx) -> list of output cells (dy,dx)."""
    cells = {}
    for dy in range(8):
        for dx in range(8):
            sy, sx = dy - 4, dx - 4
            key = (sy, sx) if (sy > 0 or (sy == 0 and sx >= 0)) else (-sy, -sx)
            cells.setdefault(key, []).append((dy, dx))
    assert len(cells) == 40
    return cells


CELLS = _canonical_cells()
ORDER = sorted(CELLS.keys(), key=lambda s: (s[0], abs(s[1])))


def _amat_np():
    """Merged vertical box-sum weights, 192 cols per sy:
      [ 0: 64)  beta=0 pair: cols j<32 -> (c4=0, win j), j>=32 -> (c4=1, j-32)
      [64:128)  beta=2 pair: cols j<32 -> (c4=2, win j), j>=32 -> (c4=3, j-32)
      [128:160) beta=1, c4=1 ; [160:192) beta=1, c4=2
    A[p, col] = 1 if superrow g=128*beta+p is in channel-group c4 and its
    h lies in vertical window i for shift sy."""
    a = np.zeros((P, 5 * 192), np.float16)
    blocks = [(0, 0, 0), (0, 1, 32), (2, 2, 64), (2, 3, 96),
              (1, 1, 128), (1, 2, 160)]
    for sy in range(5):
        for beta, c4, cb in blocks:
            base = sy * 192 + cb
            for p in range(P):
                g = 128 * beta + p
                if g // 96 != c4:
                    continue
                h = g % 96
                for i in range(NH):
                    if 0 <= h - 4 * i < 8 - sy:
                        a[p, base + i] = 1.0
    return a


def build_nc():
    nc = bacc.Bacc()
    x_dram = nc.dram_tensor("x", [C, H, W], fp32, kind="ExternalInput")
    amat_dram = nc.dram_tensor("amat", [P, 5 * 192], fp16,
                               kind="ExternalInput")
    out_dram = nc.dram_tensor("out", [40, P, NH * C16], fp32,
                              kind="ExternalOutput")
    x0_dram = nc.dram_tensor("x0scratch", [P, FD], fp16, kind="Internal")

    with TileContext(nc) as tc:
        with (
            tc.tile_pool(name="const", bufs=1) as cpool,
            tc.tile_pool(name="stage", bufs=1) as spool,
            tc.tile_pool(name="xt", bufs=1) as tpool,
            tc.tile_pool(name="q", bufs=4) as qpool,
            tc.tile_pool(name="o", bufs=4) as opool,
            tc.tile_pool(name="ps", bufs=6, space="PSUM") as ppool,
        ):
            amat_t = cpool.tile([P, 5 * 192], fp16)
            nc.sync.dma_start(amat_t, amat_dram[:, :])

            # ---- load x into staging, layout [p, (beta, c16, w)] fp32
            stage = spool.tile([P, NB * C16 * W], fp32)
            for beta, p0, p1, c4, h0 in LOAD_PIECES:
                src = x_dram[16 * c4:16 * (c4 + 1), h0:h0 + (p1 - p0), :]
                nc.sync.dma_start(
                    stage[p0:p1, beta * C16 * W:(beta + 1) * C16 * W],
                    src.rearrange("c h w -> h c w"),
                )

            # ---- X0: phase-major packed fp16, zero-padded
            x0 = tpool.tile([P, FD], fp16)
            x0v = x0.rearrange("p (b r a c) -> p b r a c", b=NB, r=4, a=APD)
            x0q = x0.rearrange("p (q a c) -> p q a c", q=NB * 4, a=APD)
            nc.gpsimd.memset(x0q[:, :, 0, :], 0.0)
            nc.gpsimd.memset(x0q[:, :, 25, :], 0.0)
            stgv = stage.rearrange("p (b c w) -> p b w c", b=NB, c=C16)
            for beta in range(NB):
                for r in range(4):
                    dst = x0v[:, beta, r, 1:25, :]
                    src = stgv[:, beta, r::4, :]
                    if (beta * 4 + r) % 2:
                        nc.vector.tensor_copy(dst, src)
                    else:
                        nc.scalar.copy(dst, src)

            # ---- vertical-shift copies T_sy (superrow g -> g+sy), bounced
            # through scratch DRAM: SBUF->SBUF DMA runs at ~25 GB/s, while
            # SBUF->DRAM->SBUF runs at full DMA rate on parallel queues.
            nc.gpsimd.dma_start(x0_dram[:, :], x0[:, :])
            tt = {0: x0}
            for sy in range(1, 5):
                t = tpool.tile([P, FD], fp16, name=f"T{sy}")
                tt[sy] = t
                nc.gpsimd.memset(t[96:P, 2 * FB:3 * FB], 0.0)
                for half in range(2):
                    nc.gpsimd.dma_start(
                        t[0:P - sy, half * 2496:(half + 1) * 2496],
                        x0_dram[sy:P, half * 2496:(half + 1) * 2496],
                    )
                nc.gpsimd.dma_start(
                    t[P - sy:P, 0:2 * FB], x0_dram[0:sy, FB:3 * FB]
                )

            # ---- per-shift: product -> col-tiled matmuls -> copy -> DMA out
            for ks, (sy, sx) in enumerate(ORDER):
                eng = nc.gpsimd if (sy, sx) in GPSIMD_SHIFTS else nc.vector
                q = qpool.tile([P, FD], fp16, tag="q")
                q3 = q.rearrange("p (b f) -> p b f", b=NB)
                x03 = x0.rearrange("p (b f) -> p b f", b=NB)
                t3 = tt[sy].rearrange("p (b f) -> p b f", b=NB)

                def prod(o0, o1, i0):
                    eng.tensor_mul(
                        q3[:, :, o0:o1],
                        x03[:, :, o0:o1],
                        t3[:, :, i0:i0 + (o1 - o0)],
                    )

                # edge pad columns skipped by the sx=+-4 trims stay zero
                # from the previous tenant of the q buffer (first tenants
                # are the full-span sy=0 shifts), so no memsets needed.
                if sx == 4:  # lambda=1 for all r; trim OOB tail (ap=25, r=3)
                    prod(0, 4 * FR - C16, C16)
                elif sx >= 0:
                    prod(0, (4 - sx) * FR, sx * FR)
                    if sx > 0:
                        prod((4 - sx) * FR, 4 * FR, C16)
                elif sx > -4:
                    s = -sx
                    prod(0, s * FR, (4 - s) * FR - C16)
                    prod(s * FR, 4 * FR, 0)
                else:  # sx == -4: lambda=-1 for all r; trim OOB head (ap=0, r=0)
                    prod(C16, FR, 0)
                    prod(FR, 4 * FR, FR - C16)

                # matmuls: 4 independent accumulation chains, one per c4
                # (PSUM rows [32*c4, +32), col-tile position 32*c4).
                # Chains never share PSUM rows, so the PE overlaps them;
                # emission order keeps adjacent mms on distinct positions.
                # (c4, beta, amat col): chain c4=1 has beta 0+1, c4=2 has 1+2
                mm_seq = [(0, 0, 0), (1, 0, 32), (2, 1, 160),
                          (3, 2, 96), (1, 1, 128), (2, 2, 64)]
                chain_first = {0: 0, 1: 1, 2: 2, 3: 3}
                chain_last = {0: 0, 1: 4, 2: 5, 3: 3}
                xlist = list(range(max(0, -sx), 8 - max(0, sx)))
                pt = ppool.tile([P, 384], fp32, tag="ps")
                for xi, xx in enumerate(xlist):
                    rx, jjx = xx & 3, xx >> 2
                    rhs_off = rx * FR + (jjx + 1) * C16
                    for mi, (c4, bb, acol) in enumerate(mm_seq):
                        nc.tensor.matmul(
                            pt[32 * c4:32 * c4 + 32, :],
                            amat_t[:, sy * 192 + acol:sy * 192 + acol + 32],
                            q3[:, bb, rhs_off:rhs_off + 384],
                            start=(xi == 0 and chain_first[c4] == mi),
                            stop=(xi == len(xlist) - 1
                                  and chain_last[c4] == mi),
                            tile_position=(0, 32 * c4),
                            skip_group_check=True,
                        )

                o_t = opool.tile([P, 384], fp32, tag="o")
                nc.scalar.copy(o_t, pt)
                dma_eng = nc.sync if ks % 2 == 0 else nc.scalar
                dma_eng.dma_start(out_dram[ks], o_t[:, 0:NH * C16])

    if not nc.is_finalized():
        nc.finalize()
    return nc


@functools.lru_cache(maxsize=1)
def _get_nc():
    return build_nc()


def _run(x, trace=False):
    amat = _amat_np()
    nc = _get_nc()
    in_maps = [
        {"x": np.ascontiguousarray(x[b]), "amat": amat} for b in range(NCORES)
    ]
    return bass_utils.run_bass_kernel_spmd(
        nc, in_maps, core_ids=list(range(NCORES)), trace=trace,
    )


def kernel(**inputs) -> np.ndarray:
    x = np.asarray(inputs["x"], dtype=np.float32)
    assert x.shape == (B, C, H, W)
    res = _run(x, trace=bool(int(os.environ.get("KERNEL_TRACE", "0"))))
    outs = np.stack([r["out"] for r in res.results])  # [B, 40, 128, (j c16)]
    blk = outs.reshape(B, 40, C4, 32, NH, C16)[:, :, :, :NH]
    blk = blk.transpose(0, 1, 2, 5, 3, 4).reshape(B, 40, C, NH, NH)
    full = np.empty((B, C, NH, NH, 8, 8), np.float32)
    for ks, key in enumerate(ORDER):
        for dy, dx in CELLS[key]:
            full[:, :, :, :, dy, dx] = blk[:, ks]
    return full


if __name__ == "__main__":
    rng = np.random.default_rng(0)
    x = rng.standard_normal((B, C, H, W), dtype=np.float32)
    y = kernel(x=x)
    print("out", y.shape, y.dtype, float(np.abs(y).max()))
